# revision 11
# baseline (speedup 1.0000x reference)
"""Causal multi-head attention (B=4, S=2048, D=1024, H=16, Hd=64) on 8 TRN2
NeuronCores.

Sharding: tensor-parallel over heads. Core c owns heads [2c, 2c+1]:
  - Wq/Wk/Wv column-sharded (rows of the [out,in] weight): each core projects
    x -> qT/kT/vT [128, S] (2 heads x 64, head-dim-major).
  - Attention per (b, h) computed entirely on-core, scoresT layout
    [keys, queries] so softmax normalization folds into matmuls.
  - Wo row-sharded: each core emits a partial [B,S,D] output; host sums the
    8 partials.

Numerics: matmuls in float32r (TF32-like, ~2e-4 relerr), softmax without
max-subtraction (scores are bounded ~|10| for this data distribution: x,W are
unit-scale gaussians and Wq is pre-scaled by 1/sqrt(Hd); exp stays well inside
fp32 range), causal mask applied post-exp as a {0,1} multiply.
"""

import os
import numpy as np
from contextlib import ExitStack

import concourse.bass as bass
import concourse.tile as tile
from concourse import bacc, mybir
from concourse.bass_utils import run_bass_kernel_spmd
from concourse.masks import make_identity

F32 = mybir.dt.float32
F32R = mybir.dt.float32r

B, S, D = 4, 2048, 1024
H, HD = 16, 64
NCORES = 8
HPC = H // NCORES          # heads per core
DH = HPC * HD              # local head dim (128)
TC = 512                   # token chunk for projections / query chunk
KS = 128                   # key strip
NEG = None                 # (mask is multiplicative post-exp)

last_exec_time_ns = None   # set by kernel() when BASS_TRACE=1


def emit(tc_ctx: tile.TileContext, ctx: ExitStack, aps: dict, b_count: int, seq: int):
    """Emit the per-core program. aps: xt [b,D,seq], wq/wk/wv [D,DH],
    wo [DH,D], mask [128, 896], out [b,seq,D]."""
    nc = tc_ctx.nc
    tc = tc_ctx
    KC = D // 128            # contraction chunks for projections
    NTC = seq // TC          # token chunks
    NQC = seq // TC          # query chunks
    NKS = seq // KS          # key strips

    xt, wq, wk, wv, wo, mask, out = (
        aps["xt"], aps["wq"], aps["wk"], aps["wv"], aps["wo"], aps["mask"], aps["out"]
    )

    wpool = ctx.enter_context(tc.tile_pool(name="wpool", bufs=1))
    xpool = ctx.enter_context(tc.tile_pool(name="xpool", bufs=2))
    qkpool = ctx.enter_context(tc.tile_pool(name="qkpool", bufs=2))
    vpool = ctx.enter_context(tc.tile_pool(name="vpool", bufs=2))
    ppool = ctx.enter_context(tc.tile_pool(name="ppool", bufs=3))
    avpool = ctx.enter_context(tc.tile_pool(name="avpool", bufs=2))
    smalls = ctx.enter_context(tc.tile_pool(name="smalls", bufs=2))

    ps_scr = ctx.enter_context(tc.tile_pool(name="ps_scr", bufs=2, space="PSUM"))
    ps_p = ctx.enter_context(tc.tile_pool(name="ps_p", bufs=2, space="PSUM"))
    ps_av = ctx.enter_context(tc.tile_pool(name="ps_av", bufs=2, space="PSUM"))

    # --- constants / weights ---
    w_sb = {}
    for name, ap in (("wq", wq), ("wk", wk), ("wv", wv)):
        t = wpool.tile([128, KC, DH], F32R, tag=name)
        nc.sync.dma_start(out=t, in_=ap.rearrange("(kc p) m -> p kc m", p=128).bitcast(F32R))
        w_sb[name] = t
    wo_sb = wpool.tile([128, D], F32R)
    nc.sync.dma_start(out=wo_sb, in_=wo.bitcast(F32R))
    mask_sb = wpool.tile([128, 896], F32R)
    nc.sync.dma_start(out=mask_sb, in_=mask.bitcast(F32R))

    ident_f = wpool.tile([128, 64], F32)
    make_identity(nc, ident_f[0:64, :])
    make_identity(nc, ident_f[64:128, :])
    ident = wpool.tile([128, 64], F32R)
    nc.vector.tensor_copy(ident, ident_f)

    ones_f = wpool.tile([128, 64], F32)
    nc.vector.memset(ones_f, 1.0)
    ones_r = wpool.tile([128, 64], F32R)
    nc.vector.tensor_copy(ones_r, ones_f)

    for b in range(b_count):
        # --- projections: qT/kT/vT [128, seq] head-dim-major ---
        qT = qkpool.tile([128, seq], F32R, tag="qT")
        kT = qkpool.tile([128, seq], F32R, tag="kT")
        vT = vpool.tile([128, seq], F32R, tag="vT")
        dst = {"wq": qT, "wk": kT, "wv": vT}
        for tcc in range(NTC):
            xt_t = xpool.tile([128, KC, TC], F32R)
            nc.sync.dma_start(
                out=xt_t,
                in_=xt[b].rearrange("(kc p) t -> p kc t", p=128)[:, :, tcc * TC:(tcc + 1) * TC].bitcast(F32R),
            )
            for name in ("wq", "wk", "wv"):
                ps = ps_scr.tile([128, TC], F32, tag="scr")
                for kc in range(KC):
                    nc.tensor.matmul(ps, w_sb[name][:, kc, :], xt_t[:, kc, :],
                                     start=(kc == 0), stop=(kc == KC - 1))
                nc.vector.tensor_copy(dst[name][:, tcc * TC:(tcc + 1) * TC], ps)

        # --- build v_ext [128, HPC, NKS, 65]: token-major v + ones column ---
        vext = vpool.tile([128, HPC, NKS, 65], F32R, tag="vext")
        for h in range(HPC):
            for ks in range(NKS):
                tr = ps_scr.tile([128, 64], F32R, tag="scr")
                nc.tensor.transpose(
                    tr, vT[h * 64:(h + 1) * 64, ks * 128:(ks + 1) * 128],
                    ident[h * 64:(h + 1) * 64, :])
                nc.vector.tensor_copy(vext[:, h, ks, 0:64], tr)
                nc.vector.tensor_copy(vext[:, h, ks, 64:65], ones_r[:, 0:1])

        # --- attention ---
        avT = avpool.tile([128, seq], F32R, tag="avT")
        for qc in range(NQC):
            nstrips = 4 * qc + 4
            pav = {h: ps_av.tile([65, TC], F32, tag="av", name=f"pav{h}")
                   for h in range(HPC)}
            for g in range(nstrips // 2):
                for h in range(HPC):
                    qh = qT[h * 64:(h + 1) * 64, qc * TC:(qc + 1) * TC]
                    pp = ps_p.tile([128, 2, TC], F32, tag="pp")
                    for j in range(2):
                        st = g * 2 + j
                        nc.tensor.matmul(pp[:, j, :],
                                         kT[h * 64:(h + 1) * 64, st * 128:(st + 1) * 128],
                                         qh, start=True, stop=True)
                    p_sb = ppool.tile([128, 2, TC], F32R, tag="p")
                    nc.scalar.activation(p_sb.rearrange("p a b -> p (a b)"),
                                         pp.rearrange("p a b -> p (a b)"),
                                         mybir.ActivationFunctionType.Exp)
                    for j in range(2):
                        st = g * 2 + j
                        r = st * 128 - qc * TC
                        if r >= 0:  # partial (diagonal) strip: mask post-exp
                            nc.vector.tensor_mul(p_sb[:, j, :], p_sb[:, j, :],
                                                 mask_sb[:, 384 - r:384 - r + TC])
                    for j in range(2):
                        st = g * 2 + j
                        nc.tensor.matmul(pav[h], vext[:, h, st, :], p_sb[:, j, :],
                                         start=(st == 0), stop=(st == nstrips - 1))
            for h in range(HPC):
                ave = smalls.tile([65, TC], F32R, tag="ave")
                nc.vector.tensor_copy(ave, pav[h])
                zb = ps_scr.tile([64, TC], F32, tag="scr")
                nc.tensor.matmul(zb, ones_r[64:65, 0:64], ave[64:65, :],
                                 start=True, stop=True)
                rz = smalls.tile([64, TC], F32R, tag="rz")
                with nc.allow_low_precision(reason="f32r holds full fp32 range; recip feeds a mul"):
                    nc.vector.reciprocal(rz, zb)
                nc.vector.tensor_mul(avT[h * 64:(h + 1) * 64, qc * TC:(qc + 1) * TC],
                                     ave[0:64, :], rz)

        # --- output projection (partial over local heads) ---
        for t16 in range(seq // 128):
            for n2 in range(D // TC):
                po = ps_scr.tile([128, TC], F32, tag="scr")
                nc.tensor.matmul(po, avT[:, t16 * 128:(t16 + 1) * 128],
                                 wo_sb[:, n2 * TC:(n2 + 1) * TC],
                                 start=True, stop=True)
                o_sb = smalls.tile([128, TC], F32, tag="o")
                nc.vector.tensor_copy(o_sb, po)
                nc.sync.dma_start(
                    out=out[b, t16 * 128:(t16 + 1) * 128, n2 * TC:(n2 + 1) * TC],
                    in_=o_sb)


def host_inputs(x, Wq, Wk, Wv, Wo, core):
    """Build the per-core input map (all float32 numpy)."""
    hs = slice(core * DH, (core + 1) * DH)
    xt = np.ascontiguousarray(np.transpose(x, (0, 2, 1)), dtype=np.float32)
    wq = np.ascontiguousarray((Wq[hs, :] * np.float32(1.0 / np.sqrt(HD))).T, dtype=np.float32)
    wk = np.ascontiguousarray(Wk[hs, :].T, dtype=np.float32)
    wv = np.ascontiguousarray(Wv[hs, :].T, dtype=np.float32)
    wo = np.ascontiguousarray(Wo[:, hs].T, dtype=np.float32)
    mask = (np.arange(896)[None, :] >= (np.arange(128)[:, None] + 384)).astype(np.float32)
    return {"xt": xt, "wq": wq, "wk": wk, "wv": wv, "wo": wo, "mask": mask}


def build_program(b_count=B, seq=S):
    nc = bacc.Bacc("TRN2", target_bir_lowering=False, debug=False,
                   num_devices=NCORES)
    aps = {
        "xt": nc.dram_tensor("xt", [b_count, D, seq], F32, kind="ExternalInput").ap(),
        "wq": nc.dram_tensor("wq", [D, DH], F32, kind="ExternalInput").ap(),
        "wk": nc.dram_tensor("wk", [D, DH], F32, kind="ExternalInput").ap(),
        "wv": nc.dram_tensor("wv", [D, DH], F32, kind="ExternalInput").ap(),
        "wo": nc.dram_tensor("wo", [DH, D], F32, kind="ExternalInput").ap(),
        "mask": nc.dram_tensor("mask", [128, 896], F32, kind="ExternalInput").ap(),
        "out": nc.dram_tensor("out", [b_count, seq, D], F32, kind="ExternalOutput").ap(),
    }
    with tile.TileContext(nc) as tcx:
        with ExitStack() as ctx:
            emit(tcx, ctx, aps, b_count, seq)
    nc.finalize()
    return nc


def _ensure_ntff_hook():
    """Register the ctypes NTFF profile hook when the image lacks
    antenv.axon_hooks (needed only for trace=True)."""
    import sys, types
    try:
        import antenv.axon_hooks  # noqa: F401
        return
    except ImportError:
        pass
    try:
        import antenv
        from trn_agent_boot.trn_boot import _ntff_profile_via_ctypes
        hook = _ntff_profile_via_ctypes("/opt/axon/libaxon_pjrt.so")
        mod = types.ModuleType("antenv.axon_hooks")
        mod.get_axon_ntff_profile_hook = lambda: hook
        mod.set_axon_ntff_profile_hook = lambda h: None
        sys.modules["antenv.axon_hooks"] = mod
        antenv.axon_hooks = mod
    except Exception:
        pass


def kernel(x, Wq, Wk, Wv, Wo):
    global last_exec_time_ns
    x = np.asarray(x, dtype=np.float32)
    Wq = np.asarray(Wq, dtype=np.float32)
    Wk = np.asarray(Wk, dtype=np.float32)
    Wv = np.asarray(Wv, dtype=np.float32)
    Wo = np.asarray(Wo, dtype=np.float32)

    nc = build_program(B, S)
    in_maps = [host_inputs(x, Wq, Wk, Wv, Wo, c) for c in range(NCORES)]
    trace = bool(os.environ.get("BASS_TRACE"))
    if trace:
        _ensure_ntff_hook()
    res = run_bass_kernel_spmd(nc, in_maps, list(range(NCORES)), trace=trace)
    last_exec_time_ns = res.exec_time_ns
    parts = [res.results[c]["out"] for c in range(NCORES)]
    acc = parts[0].astype(np.float32)
    for p in parts[1:]:
        acc = acc + p
    return acc


# revision 12
# speedup vs baseline: 1.3259x; 1.3259x over previous
"""Causal multi-head attention (B=4, S=2048, D=1024, H=16, Hd=64) on 8 TRN2
NeuronCores.

Sharding: tensor-parallel over heads. Core c owns heads [2c, 2c+1]:
  - Wq/Wk/Wv column-sharded (rows of the [out,in] weight): each core projects
    x -> qT/kT/vT [128, S] (2 heads x 64, head-dim-major).
  - Attention per (b, h) computed entirely on-core, scoresT layout
    [keys, queries] so softmax normalization folds into matmuls.
  - Wo row-sharded: each core emits a partial [B,S,D] output; host sums the
    8 partials.

Numerics: matmul operands in bf16 (fp32 PSUM accumulation), softmax without
max-subtraction (scores are bounded ~|10| for this data distribution: x,W are
unit-scale gaussians and Wq is pre-scaled by 1/sqrt(Hd); exp stays well inside
fp32 range), causal mask applied post-exp as a {0,1} multiply.
"""

import os
import numpy as np
import ml_dtypes
from contextlib import ExitStack

import concourse.bass as bass
import concourse.tile as tile
from concourse import bacc, mybir
from concourse.bass_utils import run_bass_kernel_spmd
from concourse.masks import make_identity

F32 = mybir.dt.float32
BF16 = mybir.dt.bfloat16
NPBF16 = ml_dtypes.bfloat16

B, S, D = 4, 2048, 1024
H, HD = 16, 64
NCORES = 8
HPC = H // NCORES          # heads per core
DH = HPC * HD              # local head dim (128)
TC = 512                   # token chunk for projections / query chunk
KS = 128                   # key strip

last_exec_time_ns = None   # set by kernel() when BASS_TRACE=1


def emit(tc_ctx: tile.TileContext, ctx: ExitStack, aps: dict, b_count: int, seq: int):
    """Emit the per-core program. aps: xt [b,D,seq] bf16, wq/wk/wv [D,DH] bf16,
    wo [DH,D] bf16, mask [128, 896] bf16, out [b,seq,D] f32."""
    nc = tc_ctx.nc
    tc = tc_ctx
    KC = D // 128            # contraction chunks for projections
    NTC = seq // TC          # token chunks
    NQC = seq // TC          # query chunks
    NKS = seq // KS          # key strips

    xt, wq, wk, wv, wo, mask, out = (
        aps["xt"], aps["wq"], aps["wk"], aps["wv"], aps["wo"], aps["mask"], aps["out"]
    )

    wpool = ctx.enter_context(tc.tile_pool(name="wpool", bufs=1))
    xpool = ctx.enter_context(tc.tile_pool(name="xpool", bufs=3))
    qkpool = ctx.enter_context(tc.tile_pool(name="qkpool", bufs=2))
    vpool = ctx.enter_context(tc.tile_pool(name="vpool", bufs=2))
    ppool = ctx.enter_context(tc.tile_pool(name="ppool", bufs=3))
    avpool = ctx.enter_context(tc.tile_pool(name="avpool", bufs=2))
    smalls = ctx.enter_context(tc.tile_pool(name="smalls", bufs=3))

    ps_scr = ctx.enter_context(tc.tile_pool(name="ps_scr", bufs=2, space="PSUM"))
    ps_p = ctx.enter_context(tc.tile_pool(name="ps_p", bufs=2, space="PSUM"))
    ps_av = ctx.enter_context(tc.tile_pool(name="ps_av", bufs=2, space="PSUM"))

    # --- constants / weights ---
    w_sb = {}
    for name, ap in (("wq", wq), ("wk", wk), ("wv", wv)):
        t = wpool.tile([128, KC, DH], BF16, tag=name, name=f"w_{name}")
        nc.sync.dma_start(out=t, in_=ap.rearrange("(kc p) m -> p kc m", p=128))
        w_sb[name] = t
    wo_sb = wpool.tile([128, D], BF16)
    nc.sync.dma_start(out=wo_sb, in_=wo)
    mask_sb = wpool.tile([128, 896], BF16)
    nc.sync.dma_start(out=mask_sb, in_=mask)

    ident_f = wpool.tile([128, 64], F32)
    make_identity(nc, ident_f[0:64, :])
    make_identity(nc, ident_f[64:128, :])
    ident = wpool.tile([128, 64], BF16)
    nc.vector.tensor_copy(ident, ident_f)

    ones_f = wpool.tile([128, 64], F32)
    nc.vector.memset(ones_f, 1.0)
    ones_r = wpool.tile([128, 64], BF16)
    nc.vector.tensor_copy(ones_r, ones_f)

    for b in range(b_count):
        # --- projections: qT/kT/vT [128, seq] head-dim-major ---
        qT = qkpool.tile([128, seq], BF16, tag="qT")
        kT = qkpool.tile([128, seq], BF16, tag="kT")
        vT = vpool.tile([128, seq], BF16, tag="vT")
        dst = {"wq": qT, "wk": kT, "wv": vT}
        for tcc in range(NTC):
            xt_t = xpool.tile([128, KC, TC], BF16)
            nc.sync.dma_start(
                out=xt_t,
                in_=xt[b].rearrange("(kc p) t -> p kc t", p=128)[:, :, tcc * TC:(tcc + 1) * TC],
            )
            for name in ("wq", "wk", "wv"):
                ps = ps_scr.tile([128, TC], F32, tag="scr", name=f"ps_{name}")
                for kc in range(KC):
                    nc.tensor.matmul(ps, w_sb[name][:, kc, :], xt_t[:, kc, :],
                                     start=(kc == 0), stop=(kc == KC - 1))
                nc.vector.tensor_copy(dst[name][:, tcc * TC:(tcc + 1) * TC], ps)

        # --- build v_ext [128, HPC, NKS, 65]: token-major v + ones column ---
        vext = vpool.tile([128, HPC, NKS, 65], BF16, tag="vext")
        for h in range(HPC):
            for ks in range(NKS):
                tr = ps_scr.tile([128, 64], BF16, tag="scr", name="tr")
                nc.tensor.transpose(
                    tr, vT[h * 64:(h + 1) * 64, ks * 128:(ks + 1) * 128],
                    ident[h * 64:(h + 1) * 64, :])
                nc.vector.tensor_copy(vext[:, h, ks, 0:64], tr)
                nc.vector.tensor_copy(vext[:, h, ks, 64:65], ones_r[:, 0:1])

        # --- attention ---
        avT = avpool.tile([128, seq], BF16, tag="avT")
        for qc in range(NQC):
            nstrips = 4 * qc + 4
            pav = {h: ps_av.tile([65, TC], F32, tag="av", name=f"pav{h}")
                   for h in range(HPC)}
            for g in range(nstrips // 2):
                for h in range(HPC):
                    qh = qT[h * 64:(h + 1) * 64, qc * TC:(qc + 1) * TC]
                    pp = ps_p.tile([128, 2, TC], F32, tag="pp")
                    for j in range(2):
                        st = g * 2 + j
                        nc.tensor.matmul(pp[:, j, :],
                                         kT[h * 64:(h + 1) * 64, st * 128:(st + 1) * 128],
                                         qh, start=True, stop=True)
                    p_sb = ppool.tile([128, 2, TC], BF16, tag="p")
                    nc.scalar.activation(p_sb.rearrange("p a b -> p (a b)"),
                                         pp.rearrange("p a b -> p (a b)"),
                                         mybir.ActivationFunctionType.Exp)
                    for j in range(2):
                        st = g * 2 + j
                        r = st * 128 - qc * TC
                        if r >= 0:  # partial (diagonal) strip: mask post-exp
                            nc.vector.tensor_mul(p_sb[:, j, :], p_sb[:, j, :],
                                                 mask_sb[:, 384 - r:384 - r + TC])
                    for j in range(2):
                        st = g * 2 + j
                        nc.tensor.matmul(pav[h], vext[:, h, st, :], p_sb[:, j, :],
                                         start=(st == 0), stop=(st == nstrips - 1))
            for h in range(HPC):
                ave = smalls.tile([65, TC], BF16, tag="ave")
                nc.vector.tensor_copy(ave, pav[h])
                zb = ps_scr.tile([64, TC], F32, tag="scr", name="zb")
                nc.tensor.matmul(zb, ones_r[64:65, 0:64], ave[64:65, :],
                                 start=True, stop=True)
                rz = smalls.tile([64, TC], F32, tag="rz")
                nc.vector.reciprocal(rz, zb)
                with nc.allow_low_precision(reason="attn weights tolerate bf16"):
                    nc.vector.tensor_mul(avT[h * 64:(h + 1) * 64, qc * TC:(qc + 1) * TC],
                                         ave[0:64, :], rz)

        # --- output projection (partial over local heads) ---
        for t16 in range(seq // 128):
            for n2 in range(D // TC):
                po = ps_scr.tile([128, TC], F32, tag="scr", name="po")
                nc.tensor.matmul(po, avT[:, t16 * 128:(t16 + 1) * 128],
                                 wo_sb[:, n2 * TC:(n2 + 1) * TC],
                                 start=True, stop=True)
                o_sb = smalls.tile([128, TC], F32, tag="o")
                nc.vector.tensor_copy(o_sb, po)
                nc.sync.dma_start(
                    out=out[b, t16 * 128:(t16 + 1) * 128, n2 * TC:(n2 + 1) * TC],
                    in_=o_sb)


def host_inputs(x, Wq, Wk, Wv, Wo, core, xt_bf=None):
    """Build the per-core input map."""
    hs = slice(core * DH, (core + 1) * DH)
    if xt_bf is None:
        xt_bf = np.ascontiguousarray(np.transpose(x, (0, 2, 1))).astype(NPBF16)
    wq = np.ascontiguousarray((Wq[hs, :] * np.float32(1.0 / np.sqrt(HD))).T).astype(NPBF16)
    wk = np.ascontiguousarray(Wk[hs, :].T).astype(NPBF16)
    wv = np.ascontiguousarray(Wv[hs, :].T).astype(NPBF16)
    wo = np.ascontiguousarray(Wo[:, hs].T).astype(NPBF16)
    mask = (np.arange(896)[None, :] >= (np.arange(128)[:, None] + 384)).astype(NPBF16)
    return {"xt": xt_bf, "wq": wq, "wk": wk, "wv": wv, "wo": wo, "mask": mask}


def build_program(b_count=B, seq=S):
    nc = bacc.Bacc("TRN2", target_bir_lowering=False, debug=False,
                   num_devices=NCORES)
    aps = {
        "xt": nc.dram_tensor("xt", [b_count, D, seq], BF16, kind="ExternalInput").ap(),
        "wq": nc.dram_tensor("wq", [D, DH], BF16, kind="ExternalInput").ap(),
        "wk": nc.dram_tensor("wk", [D, DH], BF16, kind="ExternalInput").ap(),
        "wv": nc.dram_tensor("wv", [D, DH], BF16, kind="ExternalInput").ap(),
        "wo": nc.dram_tensor("wo", [DH, D], BF16, kind="ExternalInput").ap(),
        "mask": nc.dram_tensor("mask", [128, 896], BF16, kind="ExternalInput").ap(),
        "out": nc.dram_tensor("out", [b_count, seq, D], F32, kind="ExternalOutput").ap(),
    }
    with tile.TileContext(nc) as tcx:
        with ExitStack() as ctx:
            emit(tcx, ctx, aps, b_count, seq)
    nc.finalize()
    return nc


def _ensure_ntff_hook():
    """Register the ctypes NTFF profile hook when the image lacks
    antenv.axon_hooks (needed only for trace=True)."""
    import sys, types
    try:
        import antenv.axon_hooks  # noqa: F401
        return
    except ImportError:
        pass
    try:
        import antenv
        from trn_agent_boot.trn_boot import _ntff_profile_via_ctypes
        hook = _ntff_profile_via_ctypes("/opt/axon/libaxon_pjrt.so")
        mod = types.ModuleType("antenv.axon_hooks")
        mod.get_axon_ntff_profile_hook = lambda: hook
        mod.set_axon_ntff_profile_hook = lambda h: None
        sys.modules["antenv.axon_hooks"] = mod
        antenv.axon_hooks = mod
    except Exception:
        pass


def kernel(x, Wq, Wk, Wv, Wo):
    global last_exec_time_ns
    x = np.asarray(x, dtype=np.float32)
    Wq = np.asarray(Wq, dtype=np.float32)
    Wk = np.asarray(Wk, dtype=np.float32)
    Wv = np.asarray(Wv, dtype=np.float32)
    Wo = np.asarray(Wo, dtype=np.float32)

    nc = build_program(B, S)
    xt_bf = np.ascontiguousarray(np.transpose(x, (0, 2, 1))).astype(NPBF16)
    in_maps = [host_inputs(x, Wq, Wk, Wv, Wo, c, xt_bf=xt_bf) for c in range(NCORES)]
    trace = bool(os.environ.get("BASS_TRACE"))
    if trace:
        _ensure_ntff_hook()
    res = run_bass_kernel_spmd(nc, in_maps, list(range(NCORES)), trace=trace)
    last_exec_time_ns = res.exec_time_ns
    parts = [res.results[c]["out"] for c in range(NCORES)]
    acc = parts[0].astype(np.float32)
    for p in parts[1:]:
        acc = acc + p
    return acc


# revision 14
# speedup vs baseline: 1.6329x; 1.2315x over previous
"""Causal multi-head attention (B=4, S=2048, D=1024, H=16, Hd=64) on 8 TRN2
NeuronCores.

Sharding: tensor-parallel over heads. Core c owns heads [2c, 2c+1]:
  - Wq/Wk/Wv column-sharded (rows of the [out,in] weight): each core projects
    x -> qT/kT/vT [128, S] (2 heads x 64, head-dim-major).
  - Attention per (b, h) computed entirely on-core, scoresT layout
    [keys, queries] so softmax normalization folds into matmuls.
  - Wo row-sharded: each core emits a partial [B,S,D] output; host sums the
    8 partials.

Numerics: matmul operands in bf16 (fp32 PSUM accumulation), softmax without
max-subtraction (scores are bounded ~|10| for this data distribution: x,W are
unit-scale gaussians and Wq is pre-scaled by 1/sqrt(Hd); exp stays well inside
fp32 range), causal mask applied post-exp as a {0,1} multiply.
"""

import os
import numpy as np
import ml_dtypes
from contextlib import ExitStack

import concourse.bass as bass
import concourse.tile as tile
from concourse import bacc, mybir
from concourse.bass_utils import run_bass_kernel_spmd
from concourse.masks import make_identity

F32 = mybir.dt.float32
BF16 = mybir.dt.bfloat16
NPBF16 = ml_dtypes.bfloat16

B, S, D = 4, 2048, 1024
H, HD = 16, 64
NCORES = 8
HPC = H // NCORES          # heads per core
DH = HPC * HD              # local head dim (128)
TC = 512                   # token chunk for projections / query chunk
KS = 128                   # key strip

last_exec_time_ns = None   # set by kernel() when BASS_TRACE=1


def emit(tc_ctx: tile.TileContext, ctx: ExitStack, aps: dict, b_count: int, seq: int):
    """Emit the per-core program. aps: xt [b,D,seq] bf16, wq/wk/wv [D,DH] bf16,
    wo [DH,D] bf16, mask [128, 896] bf16, out [b,seq,D] f32."""
    nc = tc_ctx.nc
    tc = tc_ctx
    KC = D // 128            # contraction chunks for projections
    NTC = seq // TC          # token chunks
    NQC = seq // TC          # query chunks
    NKS = seq // KS          # key strips

    xt, wq, wk, wv, wo, mask, out = (
        aps["xt"], aps["wq"], aps["wk"], aps["wv"], aps["wo"], aps["mask"], aps["out"]
    )

    wpool = ctx.enter_context(tc.tile_pool(name="wpool", bufs=1))
    xpool = ctx.enter_context(tc.tile_pool(name="xpool", bufs=3))
    qkpool = ctx.enter_context(tc.tile_pool(name="qkpool", bufs=2))
    vpool = ctx.enter_context(tc.tile_pool(name="vpool", bufs=2))
    ppool = ctx.enter_context(tc.tile_pool(name="ppool", bufs=3))
    avpool = ctx.enter_context(tc.tile_pool(name="avpool", bufs=2))
    smalls = ctx.enter_context(tc.tile_pool(name="smalls", bufs=3))

    ps_scr = ctx.enter_context(tc.tile_pool(name="ps_scr", bufs=2, space="PSUM"))
    ps_p = ctx.enter_context(tc.tile_pool(name="ps_p", bufs=2, space="PSUM"))
    ps_av = ctx.enter_context(tc.tile_pool(name="ps_av", bufs=2, space="PSUM"))

    # --- constants / weights ---
    w_sb = {}
    for name, ap in (("wq", wq), ("wk", wk), ("wv", wv)):
        t = wpool.tile([128, KC, DH], BF16, tag=name, name=f"w_{name}")
        nc.sync.dma_start(out=t, in_=ap.rearrange("(kc p) m -> p kc m", p=128))
        w_sb[name] = t
    wo_sb = wpool.tile([128, D], BF16)
    nc.sync.dma_start(out=wo_sb, in_=wo)
    mask_sb = wpool.tile([128, 896], BF16)
    nc.sync.dma_start(out=mask_sb, in_=mask)

    ident_f = wpool.tile([128, 64], F32)
    make_identity(nc, ident_f[0:64, :])
    make_identity(nc, ident_f[64:128, :])
    ident = wpool.tile([128, 64], BF16)
    nc.vector.tensor_copy(ident, ident_f)

    ones_f = wpool.tile([128, 64], F32)
    nc.vector.memset(ones_f, 1.0)
    ones_r = wpool.tile([128, 64], BF16)
    nc.vector.tensor_copy(ones_r, ones_f)

    for b in range(b_count):
        # --- projections: qT/kT/vT [128, seq] head-dim-major ---
        qT = qkpool.tile([128, seq], BF16, tag="qT")
        kT = qkpool.tile([128, seq], BF16, tag="kT")
        vT = vpool.tile([128, seq], BF16, tag="vT")
        dst = {"wq": qT, "wk": kT, "wv": vT}
        for tcc in range(NTC):
            xt_t = xpool.tile([128, KC, TC], BF16)
            nc.sync.dma_start(
                out=xt_t,
                in_=xt[b].rearrange("(kc p) t -> p kc t", p=128)[:, :, tcc * TC:(tcc + 1) * TC],
            )
            for name in ("wq", "wk", "wv"):
                ps = ps_scr.tile([128, TC], F32, tag="scr", name=f"ps_{name}")
                for kc in range(KC):
                    nc.tensor.matmul(ps, w_sb[name][:, kc, :], xt_t[:, kc, :],
                                     start=(kc == 0), stop=(kc == KC - 1))
                nc.vector.tensor_copy(dst[name][:, tcc * TC:(tcc + 1) * TC], ps)

        # --- build v_ext [128, HPC, NKS, 65]: token-major v + ones column ---
        vext = vpool.tile([128, HPC, NKS, 65], BF16, tag="vext")
        for h in range(HPC):
            for ks4 in range(NKS // 4):
                tr4 = ps_scr.tile([128, 4, 64], BF16, tag="scr", name="tr4")
                for i in range(4):
                    ks = ks4 * 4 + i
                    nc.tensor.transpose(
                        tr4[:, i, :], vT[h * 64:(h + 1) * 64, ks * 128:(ks + 1) * 128],
                        ident[h * 64:(h + 1) * 64, :])
                nc.vector.tensor_copy(vext[:, h, ks4 * 4:(ks4 + 1) * 4, 0:64], tr4)
            nc.vector.tensor_copy(vext[:, h, :, 64:65],
                                  ones_r[:, 0:1].to_broadcast([128, NKS, 1]))

        # --- attention ---
        avT = avpool.tile([128, seq], BF16, tag="avT")
        for qc in range(NQC):
            nstrips = 4 * qc + 4
            pav = {h: ps_av.tile([65, TC], F32, tag="av", name=f"pav{h}")
                   for h in range(HPC)}
            for g in range(nstrips // 2):
                for h in range(HPC):
                    qh = qT[h * 64:(h + 1) * 64, qc * TC:(qc + 1) * TC]
                    pp = ps_p.tile([128, 2, TC], F32, tag="pp")
                    for j in range(2):
                        st = g * 2 + j
                        nc.tensor.matmul(pp[:, j, :],
                                         kT[h * 64:(h + 1) * 64, st * 128:(st + 1) * 128],
                                         qh, start=True, stop=True)
                    p_sb = ppool.tile([128, 2, TC], BF16, tag="p")
                    nc.scalar.activation(p_sb.rearrange("p a b -> p (a b)"),
                                         pp.rearrange("p a b -> p (a b)"),
                                         mybir.ActivationFunctionType.Exp)
                    for j in range(2):
                        st = g * 2 + j
                        r = st * 128 - qc * TC
                        if r >= 0:  # partial (diagonal) strip: mask post-exp
                            nc.vector.tensor_mul(p_sb[:, j, :], p_sb[:, j, :],
                                                 mask_sb[:, 384 - r:384 - r + TC])
                    for j in range(2):
                        st = g * 2 + j
                        nc.tensor.matmul(pav[h], vext[:, h, st, :], p_sb[:, j, :],
                                         start=(st == 0), stop=(st == nstrips - 1))
            for h in range(HPC):
                ave = smalls.tile([65, TC], BF16, tag="ave")
                nc.vector.tensor_copy(ave, pav[h])
                zb = ps_scr.tile([64, TC], F32, tag="scr", name="zb")
                nc.tensor.matmul(zb, ones_r[64:65, 0:64], ave[64:65, :],
                                 start=True, stop=True)
                rz = smalls.tile([64, TC], F32, tag="rz")
                nc.vector.reciprocal_approx_fast(rz, zb)
                with nc.allow_low_precision(reason="attn weights tolerate bf16"):
                    nc.vector.tensor_mul(avT[h * 64:(h + 1) * 64, qc * TC:(qc + 1) * TC],
                                         ave[0:64, :], rz)

        # --- output projection (partial over local heads) ---
        for t16 in range(seq // 128):
            for n2 in range(D // TC):
                po = ps_scr.tile([128, TC], F32, tag="scr", name="po")
                nc.tensor.matmul(po, avT[:, t16 * 128:(t16 + 1) * 128],
                                 wo_sb[:, n2 * TC:(n2 + 1) * TC],
                                 start=True, stop=True)
                o_sb = smalls.tile([128, TC], F32, tag="o")
                nc.vector.tensor_copy(o_sb, po)
                nc.sync.dma_start(
                    out=out[b, t16 * 128:(t16 + 1) * 128, n2 * TC:(n2 + 1) * TC],
                    in_=o_sb)


def host_inputs(x, Wq, Wk, Wv, Wo, core, xt_bf=None):
    """Build the per-core input map."""
    hs = slice(core * DH, (core + 1) * DH)
    if xt_bf is None:
        xt_bf = np.ascontiguousarray(np.transpose(x, (0, 2, 1))).astype(NPBF16)
    wq = np.ascontiguousarray((Wq[hs, :] * np.float32(1.0 / np.sqrt(HD))).T).astype(NPBF16)
    wk = np.ascontiguousarray(Wk[hs, :].T).astype(NPBF16)
    wv = np.ascontiguousarray(Wv[hs, :].T).astype(NPBF16)
    wo = np.ascontiguousarray(Wo[:, hs].T).astype(NPBF16)
    mask = (np.arange(896)[None, :] >= (np.arange(128)[:, None] + 384)).astype(NPBF16)
    return {"xt": xt_bf, "wq": wq, "wk": wk, "wv": wv, "wo": wo, "mask": mask}


def build_program(b_count=B, seq=S):
    nc = bacc.Bacc("TRN2", target_bir_lowering=False, debug=False,
                   num_devices=NCORES)
    aps = {
        "xt": nc.dram_tensor("xt", [b_count, D, seq], BF16, kind="ExternalInput").ap(),
        "wq": nc.dram_tensor("wq", [D, DH], BF16, kind="ExternalInput").ap(),
        "wk": nc.dram_tensor("wk", [D, DH], BF16, kind="ExternalInput").ap(),
        "wv": nc.dram_tensor("wv", [D, DH], BF16, kind="ExternalInput").ap(),
        "wo": nc.dram_tensor("wo", [DH, D], BF16, kind="ExternalInput").ap(),
        "mask": nc.dram_tensor("mask", [128, 896], BF16, kind="ExternalInput").ap(),
        "out": nc.dram_tensor("out", [b_count, seq, D], F32, kind="ExternalOutput").ap(),
    }
    with tile.TileContext(nc) as tcx:
        with ExitStack() as ctx:
            emit(tcx, ctx, aps, b_count, seq)
    nc.finalize()
    return nc


def _ensure_ntff_hook():
    """Register the ctypes NTFF profile hook when the image lacks
    antenv.axon_hooks (needed only for trace=True)."""
    import sys, types
    try:
        import antenv.axon_hooks  # noqa: F401
        return
    except ImportError:
        pass
    try:
        import antenv
        from trn_agent_boot.trn_boot import _ntff_profile_via_ctypes
        hook = _ntff_profile_via_ctypes("/opt/axon/libaxon_pjrt.so")
        mod = types.ModuleType("antenv.axon_hooks")
        mod.get_axon_ntff_profile_hook = lambda: hook
        mod.set_axon_ntff_profile_hook = lambda h: None
        sys.modules["antenv.axon_hooks"] = mod
        antenv.axon_hooks = mod
    except Exception:
        pass


def kernel(x, Wq, Wk, Wv, Wo):
    global last_exec_time_ns
    x = np.asarray(x, dtype=np.float32)
    Wq = np.asarray(Wq, dtype=np.float32)
    Wk = np.asarray(Wk, dtype=np.float32)
    Wv = np.asarray(Wv, dtype=np.float32)
    Wo = np.asarray(Wo, dtype=np.float32)

    nc = build_program(B, S)
    xt_bf = np.ascontiguousarray(np.transpose(x, (0, 2, 1))).astype(NPBF16)
    in_maps = [host_inputs(x, Wq, Wk, Wv, Wo, c, xt_bf=xt_bf) for c in range(NCORES)]
    trace = bool(os.environ.get("BASS_TRACE"))
    if trace:
        _ensure_ntff_hook()
    res = run_bass_kernel_spmd(nc, in_maps, list(range(NCORES)), trace=trace)
    last_exec_time_ns = res.exec_time_ns
    parts = [res.results[c]["out"] for c in range(NCORES)]
    acc = parts[0].astype(np.float32)
    for p in parts[1:]:
        acc = acc + p
    return acc


# revision 18
# speedup vs baseline: 1.7026x; 1.0426x over previous
"""Causal multi-head attention (B=4, S=2048, D=1024, H=16, Hd=64) on 8 TRN2
NeuronCores.

Sharding: tensor-parallel over heads. Core c owns heads [2c, 2c+1]:
  - Wq/Wk/Wv column-sharded (rows of the [out,in] weight): each core projects
    x -> qT/kT/vT [128, S] (2 heads x 64, head-dim-major).
  - Attention per (b, h) computed entirely on-core, scoresT layout
    [keys, queries] so softmax normalization folds into matmuls.
  - Wo row-sharded: each core emits a partial [B,S,D] output; host sums the
    8 partials.

Numerics: matmul operands in bf16 (fp32 PSUM accumulation), softmax without
max-subtraction (scores are bounded ~|10| for this data distribution: x,W are
unit-scale gaussians and Wq is pre-scaled by 1/sqrt(Hd); exp stays well inside
fp32 range), causal mask applied post-exp as a {0,1} multiply.
"""

import os
import numpy as np
import ml_dtypes
from contextlib import ExitStack

import concourse.bass as bass
import concourse.tile as tile
from concourse import bacc, mybir
from concourse.bass_utils import run_bass_kernel_spmd
from concourse.masks import make_identity

F32 = mybir.dt.float32
BF16 = mybir.dt.bfloat16
NPBF16 = ml_dtypes.bfloat16

B, S, D = 4, 2048, 1024
H, HD = 16, 64
NCORES = 8
HPC = H // NCORES          # heads per core
DH = HPC * HD              # local head dim (128)
TC = 512                   # token chunk for projections / query chunk
KS = 128                   # key strip

last_exec_time_ns = None   # set by kernel() when BASS_TRACE=1


def emit(tc_ctx: tile.TileContext, ctx: ExitStack, aps: dict, b_count: int, seq: int):
    """Emit the per-core program. aps: xt [b,D,seq] bf16, wq/wk/wv [D,DH] bf16,
    wo [DH,D] bf16, mask [128, 896] bf16, out [b,seq,D] f32."""
    nc = tc_ctx.nc
    tc = tc_ctx
    KC = D // 128            # contraction chunks for projections
    NTC = seq // TC          # token chunks
    NQC = seq // TC          # query chunks
    NKS = seq // KS          # key strips

    xt, wq, wk, wv, wo, mask, out = (
        aps["xt"], aps["wq"], aps["wk"], aps["wv"], aps["wo"], aps["mask"], aps["out"]
    )

    wpool = ctx.enter_context(tc.tile_pool(name="wpool", bufs=1))
    xpool = ctx.enter_context(tc.tile_pool(name="xpool", bufs=4))
    qkpool = ctx.enter_context(tc.tile_pool(name="qkpool", bufs=2))
    vpool = ctx.enter_context(tc.tile_pool(name="vpool", bufs=2))
    ppool = ctx.enter_context(tc.tile_pool(name="ppool", bufs=3))
    avpool = ctx.enter_context(tc.tile_pool(name="avpool", bufs=2))
    smalls = ctx.enter_context(tc.tile_pool(name="smalls", bufs=3))

    ps_scr = ctx.enter_context(tc.tile_pool(name="ps_scr", bufs=2, space="PSUM"))
    ps_p = ctx.enter_context(tc.tile_pool(name="ps_p", bufs=2, space="PSUM"))
    ps_av = ctx.enter_context(tc.tile_pool(name="ps_av", bufs=2, space="PSUM"))

    # --- constants / weights ---
    w_sb = {}
    for name, ap in (("wq", wq), ("wk", wk), ("wv", wv)):
        t = wpool.tile([128, KC, DH], BF16, tag=name, name=f"w_{name}")
        nc.sync.dma_start(out=t, in_=ap.rearrange("(kc p) m -> p kc m", p=128))
        w_sb[name] = t
    wo_sb = wpool.tile([128, D], BF16)
    nc.sync.dma_start(out=wo_sb, in_=wo)
    mask_sb = wpool.tile([128, 896], BF16)
    nc.sync.dma_start(out=mask_sb, in_=mask)

    ident_f = wpool.tile([128, 64], F32)
    make_identity(nc, ident_f[0:64, :])
    make_identity(nc, ident_f[64:128, :])
    ident = wpool.tile([128, 64], BF16)
    nc.vector.tensor_copy(ident, ident_f)

    ones_f = wpool.tile([128, 64], F32)
    nc.vector.memset(ones_f, 1.0)
    ones_r = wpool.tile([128, 64], BF16)
    nc.vector.tensor_copy(ones_r, ones_f)

    for b in range(b_count):
        # --- projections: qT/kT/vT [128, seq] head-dim-major ---
        qT = qkpool.tile([128, seq], BF16, tag="qT")
        kT = qkpool.tile([128, seq], BF16, tag="kT")
        vT = vpool.tile([128, seq], BF16, tag="vT")
        dst = {"wq": qT, "wk": kT, "wv": vT}
        for tcc in range(NTC):
            xt_t = xpool.tile([128, KC, TC], BF16)
            xt_src = xt[b].rearrange("(kc p) t -> p kc t", p=128)
            for kc in range(KC):  # one DMA per 128-row chunk -> parallel queues
                nc.sync.dma_start(
                    out=xt_t[:, kc, :],
                    in_=xt_src[:, kc, tcc * TC:(tcc + 1) * TC],
                )
            for name in ("wq", "wk", "wv"):
                ps = ps_scr.tile([128, TC], F32, tag="scr", name=f"ps_{name}")
                for kc in range(KC):
                    nc.tensor.matmul(ps, w_sb[name][:, kc, :], xt_t[:, kc, :],
                                     start=(kc == 0), stop=(kc == KC - 1))
                nc.vector.tensor_copy(dst[name][:, tcc * TC:(tcc + 1) * TC], ps)

        # --- build v_ext [128, HPC, NKS, 65]: token-major v + ones column ---
        vext = vpool.tile([128, HPC, NKS, 65], BF16, tag="vext")
        for h in range(HPC):
            for ks4 in range(NKS // 4):
                tr4 = ps_scr.tile([128, 4, 64], BF16, tag="scr", name="tr4")
                for i in range(4):
                    ks = ks4 * 4 + i
                    nc.tensor.transpose(
                        tr4[:, i, :], vT[h * 64:(h + 1) * 64, ks * 128:(ks + 1) * 128],
                        ident[h * 64:(h + 1) * 64, :])
                nc.vector.tensor_copy(vext[:, h, ks4 * 4:(ks4 + 1) * 4, 0:64], tr4)
            nc.vector.tensor_copy(vext[:, h, :, 64:65],
                                  ones_r[:, 0:1].to_broadcast([128, NKS, 1]))

        # --- attention ---
        avT = avpool.tile([128, seq], BF16, tag="avT")
        for qc in range(NQC):
            nstrips = 4 * qc + 4
            pav = {h: ps_av.tile([65, TC], F32, tag="av", name=f"pav{h}")
                   for h in range(HPC)}
            for g in range(nstrips // 2):
                for h in range(HPC):
                    qh = qT[h * 64:(h + 1) * 64, qc * TC:(qc + 1) * TC]
                    pp = ps_p.tile([128, 2, TC], F32, tag="pp")
                    for j in range(2):
                        st = g * 2 + j
                        nc.tensor.matmul(pp[:, j, :],
                                         kT[h * 64:(h + 1) * 64, st * 128:(st + 1) * 128],
                                         qh, start=True, stop=True)
                    p_sb = ppool.tile([128, 2, TC], BF16, tag="p")
                    nc.scalar.activation(p_sb.rearrange("p a b -> p (a b)"),
                                         pp.rearrange("p a b -> p (a b)"),
                                         mybir.ActivationFunctionType.Exp)
                    for j in range(2):
                        st = g * 2 + j
                        r = st * 128 - qc * TC
                        if r >= 0:  # partial (diagonal) strip: mask post-exp
                            nc.vector.tensor_mul(p_sb[:, j, :], p_sb[:, j, :],
                                                 mask_sb[:, 384 - r:384 - r + TC])
                    for j in range(2):
                        st = g * 2 + j
                        nc.tensor.matmul(pav[h], vext[:, h, st, :], p_sb[:, j, :],
                                         start=(st == 0), stop=(st == nstrips - 1))
            for h in range(HPC):
                ave = smalls.tile([65, TC], BF16, tag="ave")
                nc.vector.tensor_copy(ave, pav[h])
                zb = ps_scr.tile([64, TC], F32, tag="scr", name="zb")
                nc.tensor.matmul(zb, ones_r[64:65, 0:64], ave[64:65, :],
                                 start=True, stop=True)
                rz = smalls.tile([64, TC], F32, tag="rz")
                nc.vector.reciprocal_approx_fast(rz, zb)
                with nc.allow_low_precision(reason="attn weights tolerate bf16"):
                    nc.vector.tensor_mul(avT[h * 64:(h + 1) * 64, qc * TC:(qc + 1) * TC],
                                         ave[0:64, :], rz)

        # --- output projection (partial over local heads) ---
        for t16 in range(seq // 128):
            for n2 in range(D // TC):
                po = ps_scr.tile([128, TC], F32, tag="scr", name="po")
                nc.tensor.matmul(po, avT[:, t16 * 128:(t16 + 1) * 128],
                                 wo_sb[:, n2 * TC:(n2 + 1) * TC],
                                 start=True, stop=True)
                o_sb = smalls.tile([128, TC], BF16, tag="o")
                nc.vector.tensor_copy(o_sb, po)
                nc.sync.dma_start(
                    out=out[b, t16 * 128:(t16 + 1) * 128, n2 * TC:(n2 + 1) * TC],
                    in_=o_sb)


def host_inputs(x, Wq, Wk, Wv, Wo, core, xt_bf=None):
    """Build the per-core input map."""
    hs = slice(core * DH, (core + 1) * DH)
    if xt_bf is None:
        xt_bf = np.ascontiguousarray(np.transpose(x, (0, 2, 1))).astype(NPBF16)
    wq = np.ascontiguousarray((Wq[hs, :] * np.float32(1.0 / np.sqrt(HD))).T).astype(NPBF16)
    wk = np.ascontiguousarray(Wk[hs, :].T).astype(NPBF16)
    wv = np.ascontiguousarray(Wv[hs, :].T).astype(NPBF16)
    wo = np.ascontiguousarray(Wo[:, hs].T).astype(NPBF16)
    mask = (np.arange(896)[None, :] >= (np.arange(128)[:, None] + 384)).astype(NPBF16)
    return {"xt": xt_bf, "wq": wq, "wk": wk, "wv": wv, "wo": wo, "mask": mask}


def build_program(b_count=B, seq=S):
    nc = bacc.Bacc("TRN2", target_bir_lowering=False, debug=False,
                   num_devices=NCORES)
    aps = {
        "xt": nc.dram_tensor("xt", [b_count, D, seq], BF16, kind="ExternalInput").ap(),
        "wq": nc.dram_tensor("wq", [D, DH], BF16, kind="ExternalInput").ap(),
        "wk": nc.dram_tensor("wk", [D, DH], BF16, kind="ExternalInput").ap(),
        "wv": nc.dram_tensor("wv", [D, DH], BF16, kind="ExternalInput").ap(),
        "wo": nc.dram_tensor("wo", [DH, D], BF16, kind="ExternalInput").ap(),
        "mask": nc.dram_tensor("mask", [128, 896], BF16, kind="ExternalInput").ap(),
        "out": nc.dram_tensor("out", [b_count, seq, D], BF16, kind="ExternalOutput").ap(),
    }
    with tile.TileContext(nc) as tcx:
        with ExitStack() as ctx:
            emit(tcx, ctx, aps, b_count, seq)
    nc.finalize()
    return nc


def _ensure_ntff_hook():
    """Register the ctypes NTFF profile hook when the image lacks
    antenv.axon_hooks (needed only for trace=True)."""
    import sys, types
    try:
        import antenv.axon_hooks  # noqa: F401
        return
    except ImportError:
        pass
    try:
        import antenv
        from trn_agent_boot.trn_boot import _ntff_profile_via_ctypes
        hook = _ntff_profile_via_ctypes("/opt/axon/libaxon_pjrt.so")
        mod = types.ModuleType("antenv.axon_hooks")
        mod.get_axon_ntff_profile_hook = lambda: hook
        mod.set_axon_ntff_profile_hook = lambda h: None
        sys.modules["antenv.axon_hooks"] = mod
        antenv.axon_hooks = mod
    except Exception:
        pass


def kernel(x, Wq, Wk, Wv, Wo):
    global last_exec_time_ns
    x = np.asarray(x, dtype=np.float32)
    Wq = np.asarray(Wq, dtype=np.float32)
    Wk = np.asarray(Wk, dtype=np.float32)
    Wv = np.asarray(Wv, dtype=np.float32)
    Wo = np.asarray(Wo, dtype=np.float32)

    nc = build_program(B, S)
    xt_bf = np.ascontiguousarray(np.transpose(x, (0, 2, 1))).astype(NPBF16)
    in_maps = [host_inputs(x, Wq, Wk, Wv, Wo, c, xt_bf=xt_bf) for c in range(NCORES)]
    trace = bool(os.environ.get("BASS_TRACE"))
    if trace:
        _ensure_ntff_hook()
    res = run_bass_kernel_spmd(nc, in_maps, list(range(NCORES)), trace=trace)
    last_exec_time_ns = res.exec_time_ns
    parts = [res.results[c]["out"] for c in range(NCORES)]
    acc = parts[0].astype(np.float32)
    for p in parts[1:]:
        acc = acc + p
    return acc
